# revision 35
# baseline (speedup 1.0000x reference)
"""MDTA (channel-attention transformer block) Trainium2 kernel, v4.

Math (zero-bias fast path; x16 = fp16(x), per-token mu/r from x16):
  G_needed = sum_t r^2 (x-mu*1)(x-mu*1)^T = G2 - u 1^T - 1 u^T + s 1 1^T
    G2 = sum r^2 x x^T = (r^2 x)^T X,  u = sum r^2 mu x,  s = 1^T u / C
  scores = wk'^T G wq'/alpha (diag 32x32 blocks), attn = softmax
  W2 = diag(g) Wv blockdiag(attn) Wf + diag(gamma),  w2s = 1^T W2
  y_t = r_t(W2^T x16_t) - r_t mu_t w2s = W2^T(x*rb) - w2s (x) rmu_row

Structure (v4 = v3 rescheduled around SBUF residency + overlap):
  Both layouts of x (xT [c, t] and x_nat [t, c]-grouped, host-staged fp16)
  are DMA'd ONCE into SBUF up front (16 MB resident) on the two HWDGE
  rings; a PE warmup burst at t=0 flips HAM to full clock.  Phase A
  (per-token sums via shifted-ones selector matmuls) runs on resident xT
  chunks as they land, in two 32-row halves so stats math / transposes /
  the Gram of half 0 overlap the stats of half 1.  The Gram rhs is the
  resident x_nat row with a 129th column that the kernel fills with mu,
  so u = sum r^2 mu x falls out of the same matmul (no N=1 matmuls).
  zr2 = x*r^2 scaling alternates DVE tensor_scalar / ACT mul (per-
  partition scalars in nat layout).  The pair all-reduce of [G2 | u]
  runs concurrently with all phase-3 prep: xts = xt * r is computed
  in place over the resident xT for all chunks (PE broadcast builds +
  DVE multiplies) while the collective is in flight.  Phase 3 then runs
  8 PSUM banks at a time (batched W2 / rank-1 weight loads), alternating
  ACT/DVE psum->fp16 copies, and writes yT out in 1 MB HWDGE DMAs.

Sharding: 8 cores = (batch 0..3) x (token half 0..1); 66 KB pair all-reduce.
Host does layout/dtype staging only (fp16 casts, the [c, t] transpose,
gamma/alpha folding, final yT.T -> fp32).
"""

import sys

import numpy as np

for _p in ("/opt/trn_rl_repo",):
    if _p not in sys.path:
        sys.path.append(_p)

import concourse.bacc as bacc
import concourse.bass as bass
import concourse.tile as tile
from concourse import mybir
from concourse.bass_utils import run_bass_kernel_spmd

B, HH, WW, C = 4, 256, 256, 128
NH, S = 4, 32
T = HH * WW
N_CORES = 8
TLOC = T // 2
EPS = 1e-5
P = 128
GRP = 4
YC = 512
C1 = C + 1          # x_nat row stride: C channels + mu slot

F32 = mybir.dt.float32
F16 = mybir.dt.float16

AF = mybir.ActivationFunctionType
OP = mybir.AluOpType
AX = mybir.AxisListType


def build_nc(tloc=TLOC, n_cores=N_CORES):
    assert tloc % (P * GRP) == 0 and tloc % YC == 0
    nc = bacc.Bacc("TRN2", target_bir_lowering=False, debug=False,
                   num_devices=n_cores)

    ngrp = tloc // (P * GRP)  # token groups of 512 (= chunks of 512)
    nyc = tloc // YC          # stats row count; == ngrp
    assert nyc == ngrp and nyc <= 64 and nyc % 2 == 0
    HQ = nyc // 2             # rows per stats half

    x_in = nc.declare_dram_parameter("x_nat", [P, ngrp * GRP * C1], F16,
                                     isOutput=False)
    xt_in = nc.declare_dram_parameter("x_tr", [C, tloc], F16, isOutput=False)
    wq_in = nc.declare_dram_parameter("wq_g", [C, C], F32, isOutput=False)
    wk_in = nc.declare_dram_parameter("wk_g", [C, C], F32, isOutput=False)
    wvT_in = nc.declare_dram_parameter("wvT4", [S, NH * C], F32, isOutput=False)
    wf_in = nc.declare_dram_parameter("wf", [C, C], F32, isOutput=False)
    dg_in = nc.declare_dram_parameter("diag_gamma", [C, C], F32, isOutput=False)
    id32_in = nc.declare_dram_parameter("ident_f32", [P, P], F32, isOutput=False)
    id16_in = nc.declare_dram_parameter("ident_f16", [P, P], F16, isOutput=False)
    w1q_in = nc.declare_dram_parameter("w1q_pk", [C, S], F32, isOutput=False)
    k1_in = nc.declare_dram_parameter("k1_col", [C, 2], F32, isOutput=False)
    hsel_in = nc.declare_dram_parameter("hsel", [NH, C], F32, isOutput=False)
    eq_in = nc.declare_dram_parameter("eqsel", [P, 2 * nyc - 1], F16,
                                      isOutput=False)
    on16_in = nc.declare_dram_parameter("ones16", [P, P], F16, isOutput=False)
    on32_in = nc.declare_dram_parameter("ones32", [P, P], F32, isOutput=False)
    yT_out = nc.declare_dram_parameter("yT16", [C, tloc], F16, isOutput=True)

    replica_groups = [[2 * b, 2 * b + 1] for b in range(n_cores // 2)]

    XDMA = 8                 # xT preload transfers (1 MB each)
    NDMA = 16                # x_nat preload transfers (~516 KB each)
    xtw = tloc // XDMA
    ndw = ngrp // NDMA

    with tile.TileContext(nc) as tc:
        with (
            tc.tile_pool(name="const", bufs=1) as const,
            tc.tile_pool(name="sqbuf", bufs=4) as sqbuf,
            tc.tile_pool(name="small", bufs=2) as small,
            tc.tile_pool(name="ybuf", bufs=2) as ybuf,
            tc.tile_pool(name="rows", bufs=2) as rows,
            tc.tile_pool(name="dram", bufs=1, space="DRAM") as dram,
        ):
            # ---- PE warmup burst (HAM -> full clock), no DMA deps ----
            wu_sb = const.tile([P, YC], F16)
            nc.vector.memset(wu_sb, 0.0)
            with tc.tile_pool(name="psW", bufs=1, space="PSUM") as psW:
                wu_ps = psW.tile([P, YC], F32, tag="wu")
                for _ in range(14):
                    nc.tensor.matmul(wu_ps, lhsT=wu_sb[:, 0:P], rhs=wu_sb,
                                     start=True, stop=True)

            # ---- resident x (both layouts) ----
            xt_res = const.tile([C, tloc], F16)
            nat = const.tile([P, ngrp, GRP * C1], F16)
            natf = nat[:].rearrange("p g x -> p (g x)")
            # alternate halves so phase A's (qq, HQ+qq) pairs arrive in order
            CPD = HQ // XDMA              # chunks per DMA (512 KB each)
            for d in range(2 * XDMA):
                q0 = (d // 2) * CPD + (HQ if d % 2 else 0)
                nc.sync.dma_start(
                    out=xt_res[:, q0 * YC:(q0 + CPD) * YC],
                    in_=xt_in[:, q0 * YC:(q0 + CPD) * YC])
            # x_nat strictly after xT on the same ring: phase A runs
            # PE-limited (warm) while x_nat streams in behind it
            nw = ngrp * GRP * C1 // NDMA
            for d in range(NDMA):
                nc.sync.dma_start(out=natf[:, d * nw:(d + 1) * nw],
                                  in_=x_in[:, d * nw:(d + 1) * nw])
            # ---- constants (SWDGE ring; gpsimd idle until collective) ----
            wq_sb = const.tile([C, C], F32)
            wk_sb = const.tile([C, C], F32)
            wvT_sb = const.tile([S, NH, C], F32)
            wf_sb = const.tile([C, C], F32)
            dg_sb = const.tile([C, C], F32)
            id32_sb = const.tile([P, P], F32)
            id16_sb = const.tile([P, P], F16)
            w1q_sb = const.tile([C, S], F32)
            k1_sb = const.tile([C, 2], F32)
            hsel_sb = const.tile([NH, C], F32)
            eq_sb = const.tile([P, 2 * nyc - 1], F16)
            on16_sb = const.tile([P, P], F16)
            on32_sb = const.tile([P, P], F32)
            nc.gpsimd.dma_start(out=id16_sb, in_=id16_in[:])
            nc.gpsimd.dma_start(out=id32_sb, in_=id32_in[:])
            nc.gpsimd.dma_start(out=eq_sb, in_=eq_in[:])
            nc.gpsimd.dma_start(out=wq_sb, in_=wq_in[:])
            nc.gpsimd.dma_start(out=wk_sb, in_=wk_in[:])
            nc.gpsimd.dma_start(out=wvT_sb,
                                in_=wvT_in[:].rearrange("s (h c) -> s h c", h=NH))
            nc.gpsimd.dma_start(out=wf_sb, in_=wf_in[:])
            nc.gpsimd.dma_start(out=dg_sb, in_=dg_in[:])
            nc.gpsimd.dma_start(out=w1q_sb, in_=w1q_in[:])
            nc.gpsimd.dma_start(out=k1_sb, in_=k1_in[:])
            nc.gpsimd.dma_start(out=hsel_sb, in_=hsel_in[:])
            nc.gpsimd.dma_start(out=on16_sb, in_=on16_in[:])
            nc.gpsimd.dma_start(out=on32_sb, in_=on32_in[:])
            eps_sb = const.tile([P, 1], F32)
            nc.vector.memset(eps_sb, EPS)

            # stats row arrays [nyc, YC] (token t = 512*q + t')
            sx_sb = const.tile([nyc, YC], F32)
            sq_sb = const.tile([nyc, YC], F32)
            scr_sb = const.tile([nyc, YC], F32)
            scr2_sb = const.tile([nyc, YC], F32)
            mu16_sb = const.tile([nyc, YC], F16)
            rmu16_sb = const.tile([nyc, YC], F16)
            r16_sb = const.tile([nyc, YC], F16)
            # column-layout r^2 (fp32, per-partition scalars for zr2)
            r2col = const.tile([P, GRP, ngrp], F32)

            ZRING = 8
            zr2 = const.tile([P, ZRING, C], F16)
            g_sb = small.tile([C, C1], F32)

            with (
                tc.tile_pool(name="psS", bufs=1, space="PSUM") as psS,
                tc.tile_pool(name="ps2", bufs=1, space="PSUM") as ps2,
                tc.tile_pool(name="psG", bufs=1, space="PSUM") as psG,
            ):
                G_ps = psG.tile([C, C1], F32, tag="g")
                nlast = ngrp * GRP - 1
                # ======== Phase A: per-token sums via PE, 4-way column-
                # tiled: one [128, YC] PSUM tile holds sx h0 / sx h1 /
                # sq h0 / sq h1 in the four 32-partition column groups, so
                # the four selector matmuls of an iteration overlap in the
                # array.  Iteration qq consumes chunks qq and HQ+qq (the
                # xT DMA order above alternates halves to match).
                sxq_ps = psS.tile([P, YC], F32, tag="sxq")
                for qq in range(HQ):
                    rhs4 = []
                    for q in (qq, HQ + qq):
                        xtq = xt_res[:, q * YC:(q + 1) * YC]
                        sqg = sqbuf.tile([C, YC], F16, name="sqg", tag="sq")
                        if q % 2 == 0:
                            nc.vector.tensor_tensor(out=sqg, in0=xtq, in1=xtq,
                                                    op=OP.mult)
                        else:
                            nc.scalar.square(out=sqg, in_=xtq)
                        rhs4.append((q, xtq, sqg))
                    for gi, (q, xtq, sqg) in enumerate(rhs4):
                        c0 = nyc - 1 - q + (HQ if q >= HQ else 0)
                        eq_v = eq_sb[:, c0:c0 + HQ]
                        st = (qq == 0)
                        sp = (qq == HQ - 1)
                        px = 32 * gi
                        nc.tensor.matmul(sxq_ps[px:px + HQ], lhsT=eq_v,
                                         rhs=xtq, start=st, stop=sp,
                                         tile_position=(0, px))
                        pq = 64 + 32 * gi
                        nc.tensor.matmul(sxq_ps[pq:pq + HQ], lhsT=eq_v,
                                         rhs=sqg, start=st, stop=sp,
                                         tile_position=(0, pq))
                # aligned copies; sq needs one partition-shift DMA (64->0)
                nc.vector.tensor_copy(out=sx_sb, in_=sxq_ps[0:nyc])
                sqs_sb = const.tile([P, YC], F32)
                nc.vector.tensor_copy(out=sqs_sb[nyc:P], in_=sxq_ps[nyc:P])
                nc.scalar.dma_start(out=sq_sb, in_=sqs_sb[nyc:P])

                # ---- batched stats math on [nyc, YC].  mu (needs only
                # sx) and its transposes run while the sq shift-DMA and
                # variance chain proceed, so only r^2 gates the Gram.
                nc.scalar.mul(out=mu16_sb, in_=sx_sb, mul=float(1.0 / C))
                nc.vector.tensor_tensor(out=scr_sb, in0=sx_sb,
                                        in1=sx_sb, op=OP.mult)
                for mh in range(2):
                    sl = slice(mh * HQ, (mh + 1) * HQ)
                    id_h16 = id16_sb[sl, sl]
                    for j in range(GRP):
                        tpm = ps2.tile([P, HQ], F16, tag="tp")
                        nc.tensor.transpose(tpm, mu16_sb[sl, j * P:(j + 1) * P],
                                            id_h16)
                        nc.vector.tensor_copy(out=nat[:, sl, j * C1 + C],
                                              in_=tpm)
                nc.vector.scalar_tensor_tensor(
                    out=scr2_sb, in0=scr_sb,
                    scalar=float(-1.0 / C), in1=sq_sb,
                    op0=OP.mult, op1=OP.add)
                nc.scalar.activation(out=scr_sb, in_=scr2_sb,
                                     func=AF.Sqrt, bias=eps_sb[0:nyc, :],
                                     scale=float(1.0 / C))
                nc.vector.reciprocal(out=scr2_sb, in_=scr_sb)
                nc.vector.tensor_tensor(out=scr_sb, in0=scr2_sb,
                                        in1=scr2_sb, op=OP.mult)
                nc.vector.tensor_tensor(out=rmu16_sb, in0=mu16_sb,
                                        in1=scr2_sb, op=OP.mult)
                nc.scalar.copy(out=r16_sb, in_=scr2_sb)

                # r^2 -> r2col (fp32 column layout for zr2)
                for mh in range(2):
                    sl = slice(mh * HQ, (mh + 1) * HQ)
                    id_h32 = id32_sb[sl, sl]
                    for j in range(GRP):
                        tpj = ps2.tile([P, HQ], F32, tag="tp")
                        nc.tensor.transpose(tpj, scr_sb[sl, j * P:(j + 1) * P],
                                            id_h32)
                        nc.vector.tensor_copy(out=r2col[:, j, sl], in_=tpj)

                # ==== Gram: G2 += (r^2 x)^T [x | mu] ====
                for g in range(ngrp):
                    for j in range(GRP):
                        i = g * GRP + j
                        r = i % ZRING
                        xnj = nat[:, g, j * C1:j * C1 + C]
                        if i % 4 == 3:
                            nc.scalar.mul(out=zr2[:, r], in_=xnj,
                                          mul=r2col[:, j, g:g + 1])
                        else:
                            nc.vector.tensor_scalar_mul(
                                out=zr2[:, r], in0=xnj,
                                scalar1=r2col[:, j, g:g + 1])
                        nc.tensor.matmul(G_ps, lhsT=zr2[:, r],
                                         rhs=nat[:, g, j * C1:(j + 1) * C1],
                                         start=(i == 0), stop=(i == nlast))

                nc.vector.tensor_copy(out=g_sb, in_=G_ps)

            # ============ all-reduce [G2 | u] ============
            g_in_d = dram.tile([C, C1], F32)
            g_out_d = dram.tile([C, C1], F32)
            nc.gpsimd.dma_start(out=g_in_d, in_=g_sb)
            nc.gpsimd.collective_compute(
                "AllReduce", OP.add, replica_groups=replica_groups,
                ins=[g_in_d[:].opt()], outs=[g_out_d[:].opt()])

            # -- overlap: xts = xt * r, in place over resident xT.
            # First half before the softmax block (fills the collective
            # window); second half after it (overlaps early phase 3).
            RB = 8            # chunks per row-remap block

            def xts_block(psR, b):
                rt = rows.tile([1, RB * YC], F16, name="rt", tag="rt")
                nc.scalar.dma_start(out=rt,
                                    in_=r16_sb[b * RB:(b + 1) * RB, :])
                for k in range(RB):
                    q = b * RB + k
                    tsl = slice(q * YC, (q + 1) * YC)
                    rb_ps = psR.tile([C, YC], F32, tag="rb")
                    nc.tensor.matmul(rb_ps, lhsT=on16_sb[0:1, :],
                                     rhs=rt[0:1, k * YC:(k + 1) * YC],
                                     start=True, stop=True)
                    rb16 = sqbuf.tile([C, YC], F16, name="rb16", tag="rb")
                    nc.scalar.copy(out=rb16, in_=rb_ps)
                    nc.vector.tensor_tensor(out=xt_res[:, tsl],
                                            in0=xt_res[:, tsl],
                                            in1=rb16, op=OP.mult)

            with tc.tile_pool(name="psR", bufs=4, space="PSUM") as psR:
                for b in range(nyc // RB):
                    xts_block(psR, b)

            gs_sb = small.tile([C, C1], F32)
            nc.gpsimd.dma_start(out=gs_sb, in_=g_out_d)

            # ============ Phase 2: scores + softmax + W2 ============
            with (
                tc.tile_pool(name="ps3", bufs=1, space="PSUM") as ps2,
                tc.tile_pool(name="psF", bufs=1, space="PSUM") as psF,
            ):
                fil_ps = psF.tile([P, YC], F32, tag="fil")

                def filler(n):
                    # independent matmuls that keep the PE HAM-warm while
                    # the serial softmax chain waits on ACT/DVE/collective
                    for _ in range(n):
                        nc.tensor.matmul(fil_ps, lhsT=wu_sb[:, 0:P],
                                         rhs=wu_sb, start=True, stop=True)

                filler(28)     # bridge the all-reduce wait
                u_ap = gs_sb[:, C:C + 1]
                s1_ps = ps2.tile([C, C], F32, tag="mm")
                nc.tensor.matmul(s1_ps, lhsT=gs_sb[:, 0:C], rhs=wq_sb,
                                 start=True, stop=True)   # G symmetric
                s1_sb = small.tile([C, C], F32)
                nc.scalar.copy(out=s1_sb, in_=s1_ps)
                filler(2)
                sc_ps = ps2.tile([C, C], F32, tag="mm")
                nc.tensor.matmul(sc_ps, lhsT=wk_sb, rhs=s1_sb, start=True,
                                 stop=True)
                spk = small.tile([P, S], F32)
                for h in range(NH):
                    nc.scalar.copy(out=spk[h * S:(h + 1) * S, :],
                                   in_=sc_ps[h * S:(h + 1) * S,
                                             h * S:(h + 1) * S])

                a_ps = ps2.tile([C, 1], F32, tag="sm")
                nc.tensor.matmul(a_ps, lhsT=wk_sb, rhs=u_ap, start=True,
                                 stop=True)
                a_sb = small.tile([C, 1], F32)
                nc.vector.tensor_copy(out=a_sb, in_=a_ps)
                bc_ps = ps2.tile([C, 1], F32, tag="sm")
                nc.tensor.matmul(bc_ps, lhsT=wq_sb, rhs=u_ap, start=True,
                                 stop=True)
                bc_sb = small.tile([C, 1], F32)
                nc.scalar.copy(out=bc_sb, in_=bc_ps)
                su_ps = ps2.tile([1, 1], F32, tag="sm")
                nc.tensor.matmul(su_ps, lhsT=u_ap, rhs=on32_sb[:, 0:1],
                                 start=True, stop=True)
                su_sb = small.tile([1, 1], F32)
                nc.scalar.copy(out=su_sb, in_=su_ps)
                filler(2)
                sc_col_ps = ps2.tile([C, 1], F32, tag="sm")
                nc.tensor.matmul(sc_col_ps, lhsT=on32_sb[0:1, :], rhs=su_sb,
                                 start=True, stop=True)
                scol_sb = small.tile([C, 1], F32)
                nc.scalar.mul(out=scol_sb, in_=sc_col_ps, mul=float(1.0 / C))

                bT_ps = ps2.tile([1, C], F32, tag="sm")
                nc.tensor.transpose(bT_ps, bc_sb, id32_sb)
                bT_sb = small.tile([1, C], F32)
                nc.scalar.copy(out=bT_sb, in_=bT_ps)
                bT4_sb = small.tile([NH, S], F32)
                nc.scalar.dma_start(out=bT4_sb, in_=bT_sb)
                filler(2)
                bpk_ps = ps2.tile([C, S], F32, tag="sm")
                nc.tensor.matmul(bpk_ps, lhsT=hsel_sb, rhs=bT4_sb,
                                 start=True, stop=True)

                tmp_sb = small.tile([C, 1], F32)
                nc.vector.scalar_tensor_tensor(
                    out=tmp_sb, in0=scol_sb, scalar=k1_sb[:, 0:1], in1=a_sb,
                    op0=OP.mult, op1=OP.subtract)             # s*k1 - a
                s1c = small.tile([P, S], F32)
                nc.vector.scalar_tensor_tensor(
                    out=s1c, in0=w1q_sb, scalar=tmp_sb, in1=spk,
                    op0=OP.mult, op1=OP.add)
                scor = small.tile([P, S], F32)
                nc.vector.scalar_tensor_tensor(
                    out=scor, in0=bpk_ps, scalar=k1_sb[:, 1:2], in1=s1c,
                    op0=OP.mult, op1=OP.add)

                mx = small.tile([P, 1], F32)
                nc.vector.reduce_max(mx, scor, AX.X)
                nmx = small.tile([P, 1], F32)
                nc.vector.tensor_scalar_mul(out=nmx, in0=mx, scalar1=-1.0)
                sh = small.tile([P, S], F32)
                nc.vector.tensor_scalar(out=sh, in0=scor, scalar1=nmx,
                                        scalar2=-87.0, op0=OP.add, op1=OP.max)
                ex = small.tile([P, S], F32)
                es = small.tile([P, 1], F32)
                nc.scalar.activation(out=ex, in_=sh, func=AF.Exp,
                                     bias=0.0, scale=1.0, accum_out=es)
                ri = small.tile([P, 1], F32)
                nc.vector.reciprocal(out=ri, in_=es)
                at = small.tile([P, S], F32)
                nc.vector.tensor_scalar_mul(out=at, in0=ex, scalar1=ri)
                at4 = small.tile([S, NH, S], F32)
                for h in range(NH):
                    nc.scalar.dma_start(out=at4[:, h, :],
                                        in_=at[h * S:(h + 1) * S, :])

                filler(10)     # softmax DVE/ACT chain + at4 remap wait
                u2_ps = ps2.tile([C, C], F32, tag="mm")
                for h in range(NH):
                    nc.tensor.matmul(u2_ps[:, h * S:(h + 1) * S],
                                     lhsT=wvT_sb[:, h, :], rhs=at4[:, h, :],
                                     start=True, stop=True)
                u2_sb = small.tile([C, C], F32)
                nc.scalar.copy(out=u2_sb, in_=u2_ps)
                filler(2)
                ut_ps = ps2.tile([C, C], F32, tag="mm")
                nc.tensor.transpose(ut_ps, u2_sb, id32_sb)
                ut_sb = small.tile([C, C], F32)
                nc.scalar.copy(out=ut_sb, in_=ut_ps)
                filler(2)
                w2_ps = ps2.tile([C, C], F32, tag="mm")
                nc.tensor.matmul(w2_ps, lhsT=ut_sb, rhs=wf_sb, start=True,
                                 stop=True)
                w2_sb = small.tile([C, C], F16)
                nc.vector.tensor_tensor(out=w2_sb, in0=w2_ps, in1=dg_sb,
                                        op=OP.add)
                ws_ps = ps2.tile([1, C], F32, tag="sm")
                nc.tensor.matmul(ws_ps, lhsT=on16_sb[:, 0:1], rhs=w2_sb,
                                 start=True, stop=True)
                nws_sb = small.tile([1, C], F16)
                nc.vector.tensor_scalar_mul(out=nws_sb, in0=ws_ps, scalar1=-1.0)
                # column form of -w2s (per-partition scalar for the STT path)
                wsc_ps = ps2.tile([C, 1], F32, tag="sm")
                nc.tensor.matmul(wsc_ps, lhsT=w2_sb, rhs=on16_sb[:, 0:1],
                                 start=True, stop=True)
                nwsc_sb = small.tile([C, 1], F32)
                nc.vector.tensor_scalar_mul(out=nwsc_sb, in0=wsc_ps,
                                            scalar1=-1.0)

            # ====== Phase 3: yp = W2^T xts - w2s (x) rmu; fp16 out ======
            # Blocks < NST use a DMA-broadcast rmu tile + one DVE
            # scalar_tensor_tensor per chunk (y16 = rmub*(-w2s) + yp):
            # no rank-1 matmul, no separate copy.  Later blocks use the
            # rank-1 matmul + ACT copies, balancing PE/DVE/ACT.
            YB = 8            # PSUM banks per block
            nyb = nyc // YB
            NST = nyb // 2
            rmu_d = dram.tile([1, NST * YB * YC], F16)
            nc.scalar.dma_start(out=rmu_d, in_=rmu16_sb[0:NST * YB, :])
            with tc.tile_pool(name="psY", bufs=YB, space="PSUM") as psY:
                for blk in range(nyb):
                    bsl = slice(blk * YB * YC, (blk + 1) * YB * YC)
                    if blk < NST:
                        HB = YB // 2 * YC
                        rmbs = []
                        for hh in range(2):
                            rmb = rows.tile([C, HB], F16, name="rmb",
                                            tag="rmb")
                            sap = rmu_d[0:1, blk * 2 * HB + hh * HB:
                                        blk * 2 * HB + (hh + 1) * HB]
                            nc.scalar.dma_start(
                                out=rmb,
                                in_=bass.AP(sap.tensor, sap.offset,
                                            [[0, C]] + list(sap.ap)[1:]))
                            rmbs.append(rmb)
                    else:
                        rmt = rows.tile([1, YB * YC], F16, name="rmt",
                                        tag="rt")
                        nc.scalar.dma_start(
                            out=rmt, in_=rmu16_sb[blk * YB:(blk + 1) * YB, :])
                    yps = []
                    for k in range(YB):
                        q = blk * YB + k
                        tsl = slice(q * YC, (q + 1) * YC)
                        yp = psY.tile([C, YC], F32, tag="y")
                        nc.tensor.matmul(yp, lhsT=w2_sb, rhs=xt_res[:, tsl],
                                         start=True, stop=(blk < NST))
                        yps.append(yp)
                    if blk >= NST:
                        for k in range(YB):
                            nc.tensor.matmul(
                                yps[k], lhsT=nws_sb,
                                rhs=rmt[0:1, k * YC:(k + 1) * YC],
                                start=False, stop=True)
                    HB2 = YB // 2 * YC
                    for hh in range(2):
                        y16 = ybuf.tile([C, HB2], F16, name="y16",
                                        tag="y16")
                        for kk in range(YB // 2):
                            k = hh * (YB // 2) + kk
                            hsl = slice(kk * YC, (kk + 1) * YC)
                            if blk < NST:
                                nc.vector.scalar_tensor_tensor(
                                    out=y16[:, hsl], in0=rmbs[hh][:, hsl],
                                    scalar=nwsc_sb, in1=yps[k],
                                    op0=OP.mult, op1=OP.add)
                            else:
                                nc.scalar.copy(out=y16[:, hsl], in_=yps[k])
                        o0 = blk * YB * YC + hh * HB2
                        nc.sync.dma_start(out=yT_out[:, o0:o0 + HB2],
                                          in_=y16)

    nc.compile()
    return nc


def _numpy_reference(x, gamma, beta, Wq, bq, Wk, bk, Wv, bv, Wf, bf, alpha):
    """Fallback for inputs outside the zero-bias fast path."""
    Bx, Hx, Wx, Cx = x.shape
    t = Hx * Wx
    nh = NH
    s = Cx // nh
    xf = x.reshape(Bx, t, Cx).astype(np.float64)
    mu = xf.mean(-1, keepdims=True)
    var = ((xf - mu) ** 2).mean(-1, keepdims=True)
    xn = (xf - mu) / np.sqrt(var + EPS) * gamma + beta
    Q = (xn @ Wq + bq).reshape(Bx, t, nh, s)
    K = (xn @ Wk + bk).reshape(Bx, t, nh, s)
    V = (xn @ Wv + bv).reshape(Bx, t, nh, s)
    scores = np.einsum("bthi,bthj->bhij", K, Q) / float(alpha)
    scores = scores - scores.max(-1, keepdims=True)
    e = np.exp(scores)
    attn = e / e.sum(-1, keepdims=True)
    out = np.einsum("bthi,bhij->bthj", V, attn).reshape(Bx, t, Cx)
    y = out @ Wf + bf + xn
    return y.reshape(Bx, Hx, Wx, Cx).astype(np.float32)


def make_in_maps(inputs, tloc=TLOC, n_cores=N_CORES):
    x = np.asarray(inputs["x"], dtype=np.float32)
    gamma = np.asarray(inputs["gamma"], dtype=np.float32)
    Wq = np.asarray(inputs["Wq"], dtype=np.float32)
    Wk = np.asarray(inputs["Wk"], dtype=np.float32)
    Wv = np.asarray(inputs["Wv"], dtype=np.float32)
    Wf = np.ascontiguousarray(np.asarray(inputs["Wf"], dtype=np.float32))
    inv_alpha = (1.0 / float(np.asarray(inputs["alpha"]))
                 if "alpha" in inputs else 1.0)

    wq_g = np.ascontiguousarray(gamma[:, None] * Wq * inv_alpha)
    wk_g = np.ascontiguousarray(gamma[:, None] * Wk)
    wv_g = gamma[:, None] * Wv
    wvT4 = np.ascontiguousarray(
        wv_g.T.reshape(NH, S, C).transpose(1, 0, 2).reshape(S, NH * C))
    diag_g = np.ascontiguousarray(np.diag(gamma).astype(np.float32))
    ident_f32 = np.eye(P, dtype=np.float32)
    ident_f16 = np.eye(P, dtype=np.float16)

    w1q = wq_g.sum(axis=0)
    w1q_pk = np.repeat(w1q.reshape(NH, S), S, axis=0).astype(np.float32)
    k1 = wk_g.sum(axis=0)
    k1_col = np.stack([k1, -k1], axis=1).astype(np.float32)
    hsel = (np.arange(C)[None, :] // S == np.arange(NH)[:, None]
            ).astype(np.float32)
    nyc = tloc // YC
    eqsel = np.zeros((P, 2 * nyc - 1), np.float16)
    eqsel[:, nyc - 1] = 1.0     # E_q = eqsel[:, nyc-1-q : 2*nyc-1-q]
    ones16 = np.ones((P, P), np.float16)
    ones32 = np.ones((P, P), np.float32)

    x16 = x.reshape(n_cores, tloc, C).astype(np.float16)
    ngrp = tloc // (P * GRP)
    # x_nat, partition-major, with a zero 129th column per token (the
    # kernel fills it with mu): [cores, P, ngrp * GRP * C1]
    xg = x16.reshape(n_cores, ngrp, GRP, P, C).transpose(0, 3, 1, 2, 4)
    xpad = np.zeros((n_cores, P, ngrp, GRP, C1), np.float16)
    xpad[..., :C] = xg
    x_nat = np.ascontiguousarray(
        xpad.reshape(n_cores, P, ngrp * GRP * C1))
    x_tr = np.ascontiguousarray(x16.transpose(0, 2, 1))

    shared = dict(wq_g=wq_g, wk_g=wk_g, wvT4=wvT4, wf=Wf, diag_gamma=diag_g,
                  ident_f32=ident_f32, ident_f16=ident_f16,
                  w1q_pk=np.ascontiguousarray(w1q_pk),
                  k1_col=np.ascontiguousarray(k1_col),
                  hsel=np.ascontiguousarray(hsel), eqsel=eqsel,
                  ones16=ones16, ones32=ones32)
    return [dict(shared, x_nat=x_nat[i], x_tr=x_tr[i]) for i in range(n_cores)]


_NC_CACHE = {}


def kernel(**inputs) -> np.ndarray:
    zero = lambda k: not np.any(np.asarray(inputs[k]))
    if not (zero("beta") and zero("bq") and zero("bk") and zero("bv")
            and zero("bf")):
        return _numpy_reference(**{k: np.asarray(v) for k, v in inputs.items()})

    key = ("v4", TLOC, N_CORES)
    if key not in _NC_CACHE:
        _NC_CACHE[key] = build_nc(TLOC, N_CORES)
    nc = _NC_CACHE[key]

    in_maps = make_in_maps(inputs)
    res = run_bass_kernel_spmd(nc, in_maps, core_ids=list(range(N_CORES)))
    yT = [res.results[i]["yT16"] for i in range(N_CORES)]
    y = np.concatenate([t.T for t in yT], axis=0).astype(np.float32)
    return np.ascontiguousarray(y.reshape(B, HH, WW, C))
